# revision 1
# baseline (speedup 1.0000x reference)
"""Trainium2 Bass kernel for a PointNet++-style feature-propagation decoder
(4 stages of kNN(k=3) inverse-distance-weighted feature interpolation).

Sharding: batch b = core//2 (data parallel over B=4), and the finest stage's
8192 query points split in half across each core pair (point parallel along N
per the sharding hint). Stages 0-2 are duplicated within a pair (cheap);
stage 3 dominates and is n-split. Output rows 0:64 are the x0 passthrough,
assembled on the host.

Per-core device pipeline per stage:
  - negated squared distances via one K=5 PE matmul per 128-query tile:
      A = [ax, ay, az, -|a|^2, -1], B = [2bx, 2by, 2bz, 1, |b|^2], A.B = -dist
  - top-3 neighbors via DVE max (top-8) + max_index
  - inverse-distance weights on DVE
  - feature gather via SWDGE indirect DMA (row gather from a DRAM source
    table, one row per partition), weighted 3-way combine via
    scalar_tensor_tensor (per-partition scalar FMA)
  - stage output rows are DMA'd into the next stage's DRAM source table;
    the final stage is transposed back to [D, N] layout via PE transposes.
"""

import numpy as np

P = 128
KNN = 3
EPS = 1e-8

B = 4
NS = [8192, 2048, 512, 128, 32]  # points per level, finest -> coarsest
CS = [64, 128, 256, 512, 1024]   # feature channels per level

_CACHED = {"nc": None, "key": None}


def _build_program(ns, cs, n_half, split_waits=True):
    """Trace the per-core Bass program. ns/cs as in reference (finest first).
    n_half: number of finest-level query points this core handles."""
    import contextlib

    import concourse.bass as bass
    import concourse.mybir as mybir
    import concourse.tile as tile
    from concourse.bass import IndirectOffsetOnAxis
    from concourse.masks import make_identity

    _patch_tile_drain()

    f32 = mybir.dt.float32
    u32 = mybir.dt.uint32
    Alu = mybir.AluOpType
    Axis = mybir.AxisListType

    # stage s: fine level (3-s) [half of level 0 for s=3], coarse level (4-s)
    # d2[s]: width of the source table gathered at stage s
    d2 = [cs[4]]
    for s in range(1, 4):
        d2.append(cs[4 - s] + d2[s - 1])
    stages = []
    for s in range(4):
        nf = ns[3 - s] if s < 3 else n_half
        stages.append(dict(nf=nf, S=ns[4 - s], d2=d2[s],
                           cx=(cs[3 - s] if s < 3 else None)))

    nc = bass.Bass("TRN2")

    # ---- external inputs (per core) ----
    xt = {}   # xyz transposed [3, N]
    xr = {}   # xyz rows [N, 3]
    for i, n in enumerate(ns):
        nn_ = n_half if i == 0 else n
        xt[i] = nc.dram_tensor(f"xt{i}", [3, nn_], f32, kind="ExternalInput")
        xr[i] = nc.dram_tensor(f"xr{i}", [nn_, 3], f32, kind="ExternalInput")
    feat = {}
    for i in range(1, 5):
        feat[i] = nc.dram_tensor(f"f{i}", [cs[i], ns[i]], f32, kind="ExternalInput")

    # ---- external output: interp part of final stage, [d2[3], n_half] ----
    oi = nc.dram_tensor("oi", [d2[3], n_half], f32, kind="ExternalOutput")

    # ---- internal DRAM source tables ----
    tsrc = []
    for s in range(4):
        tsrc.append(nc.dram_tensor(f"tsrc{s}", [stages[s]["S"], stages[s]["d2"]], f32))
    # scratch for reshaping row-layout norms into free-layout matmul rows
    nscr = {s: (nc.dram_tensor(f"nscrf{s}", [stages[s]["nf"]], f32),
                nc.dram_tensor(f"nscrc{s}", [max(stages[s]["S"], P)], f32))
            for s in range(4)}

    with tile.TileContext(nc) as tc, contextlib.ExitStack() as ctx:
        cpool = ctx.enter_context(tc.tile_pool(name="const", bufs=1))
        inpool = ctx.enter_context(tc.tile_pool(name="in", bufs=1))
        abpool = ctx.enter_context(tc.tile_pool(name="ab", bufs=1))
        ndpool = ctx.enter_context(tc.tile_pool(name="nd", bufs=3))
        smpool = ctx.enter_context(tc.tile_pool(name="sm", bufs=2))
        gpool = ctx.enter_context(tc.tile_pool(name="g", bufs=2))
        rpool = ctx.enter_context(tc.tile_pool(name="r", bufs=2))
        xtp = ctx.enter_context(tc.tile_pool(name="xtp", bufs=2))
        ps_nd = ctx.enter_context(tc.tile_pool(name="ps_nd", bufs=2, space="PSUM"))
        ps_tp = ctx.enter_context(tc.tile_pool(name="ps_tp", bufs=4, space="PSUM"))

        ident = cpool.tile([P, P], f32, tag="ident")
        make_identity(nc, ident[:])
        # constant rows (engines can't start at odd partitions; build at
        # partition 0 and DMA into place)
        maxn = max(st["nf"] for st in stages)
        cones = cpool.tile([1, maxn], f32, tag="cones")
        nc.vector.memset(cones[:], 1.0)
        cneg = cpool.tile([1, maxn], f32, tag="cneg")
        nc.vector.memset(cneg[:], -1.0)

        # ---------- build x^T parts of the source tables ----------
        def xt_into_table(fi, table, nrows, ncols):
            """write feat[fi]^T ([nrows points, ncols feats]) into
            table[:, 0:ncols] via PE transposes."""
            cchunks = (ncols + P - 1) // P
            rchunks = (nrows + P - 1) // P
            fsb = inpool.tile([min(ncols, P), cchunks * nrows], f32,
                              tag="fsb")
            src = feat[fi][:, :]
            if ncols > P:
                nc.sync.dma_start(
                    fsb[:].rearrange("p (cc n) -> p cc n", cc=cchunks),
                    src.rearrange("(cc p) n -> p cc n", p=P))
            else:
                nc.sync.dma_start(fsb[:ncols, :nrows], src)
            for t in range(rchunks):
                rows = min(P, nrows - t * P)
                xtt = xtp.tile([P, cchunks * P], f32, tag="xtt")
                for cc in range(cchunks):
                    c0 = cc * P
                    cw = min(P, ncols - c0)
                    pst = ps_tp.tile([P, P], f32, tag="pst")
                    nc.tensor.transpose(
                        pst[:rows, :cw],
                        fsb[:cw, cc * nrows + t * P: cc * nrows + t * P + rows],
                        ident[:])
                    nc.scalar.copy(xtt[:rows, c0:c0 + cw], pst[:rows, :cw])
                nc.sync.dma_start(table[t * P:t * P + rows, 0:ncols],
                                  xtt[:rows, :ncols])

        xt_into_table(4, tsrc[0], stages[0]["S"], cs[4])   # x4^T -> T0
        xt_into_table(3, tsrc[1], stages[1]["S"], cs[3])   # x3^T -> T1
        xt_into_table(2, tsrc[2], stages[2]["S"], cs[2])   # x2^T -> T2
        xt_into_table(1, tsrc[3], stages[3]["S"], cs[1])   # x1^T -> T3

        # ---------- row-layout squared norms ----------
        def row_norms(xr_dram, n, negate, tag):
            """[P, T] tile holding (+-)|p|^2 of n points, point (t*128+p) at
            [p, t] (or [p, 0] for p < n when n < 128)."""
            if n >= P:
                T = n // P
                rx = smpool.tile([P, T * 3], f32, tag=f"rx{tag}")
                nc.sync.dma_start(
                    rx[:].rearrange("p (t c) -> p t c", c=3),
                    xr_dram[:, :].rearrange("(t p) c -> p t c", p=P))
                pdim = P
            else:
                T = 1
                rx = smpool.tile([P, 3], f32, tag=f"rx{tag}")
                nc.sync.dma_start(rx[:n, :], xr_dram[:, :])
                pdim = n
            sq = smpool.tile([P, T * 3], f32, tag=f"sq{tag}")
            nc.vector.tensor_tensor(sq[:pdim, :T * 3], rx[:pdim, :T * 3],
                                    rx[:pdim, :T * 3], op=Alu.mult)
            nrm = smpool.tile([P, T], f32, tag=f"nrm{tag}")
            nc.vector.tensor_reduce(
                nrm[:pdim, :T],
                sq[:pdim, :T * 3].rearrange("p (t c) -> p t c", c=3),
                axis=Axis.X, op=Alu.add)
            if negate:
                nc.vector.tensor_scalar_mul(nrm[:pdim, :T], nrm[:pdim, :T], -1.0)
            return nrm, T

        # ---------- stages ----------
        for s, st in enumerate(stages):
            nf, S, D2 = st["nf"], st["S"], st["d2"]
            T = nf // P
            fine_lvl = 3 - s if s < 3 else 0
            coarse_lvl = 4 - s

            # A5 [5, nf]: rows 0-2 xt_fine, row3 -|a|^2, row4 -1
            a5 = abpool.tile([5, nf], f32, tag="a5")
            nc.sync.dma_start(a5[0:3, :], xt[fine_lvl][:, :])
            nfin, Tf = row_norms(xr[fine_lvl], nf, negate=True, tag="f")
            nc.sync.dma_start(
                nscr[s][0][:].rearrange("(t j) -> j t", j=P), nfin[:, :Tf])
            nc.sync.dma_start(a5[3:4, :], nscr[s][0][:])
            nc.sync.dma_start(a5[4:5, :], cneg[:, :nf])

            # B5 [5, S]: rows 0-2 2*xt_coarse, row3 1, row4 +|b|^2
            b5 = abpool.tile([5, S], f32, tag="b5")
            nc.sync.dma_start(b5[0:3, :], xt[coarse_lvl][:, :])
            nc.vector.tensor_scalar_mul(b5[0:3, :], b5[0:3, :], 2.0)
            nc.sync.dma_start(b5[3:4, :], cones[:, :S])
            ncr, Tc = row_norms(xr[coarse_lvl], S, negate=False, tag="c")
            if S >= P:
                nc.sync.dma_start(
                    nscr[s][1][:S].rearrange("(t j) -> j t", j=P),
                    ncr[:, :Tc])
                nc.sync.dma_start(b5[4:5, :], nscr[s][1][:S])
            else:
                nc.sync.dma_start(nscr[s][1][:S], ncr[:S, :1])
                nc.sync.dma_start(b5[4:5, :], nscr[s][1][:S])

            # ---- block-pipelined dist+top3 / weights / gather+combine ----
            maxb = smpool.tile([P, T * 8], f32, tag=f"maxb{s}")
            idxb = smpool.tile([P, T * 8], u32, tag=f"idxb{s}")
            dbuf = smpool.tile([P, T * KNN], f32, tag=f"dbuf{s}")
            wraw = smpool.tile([P, T * KNN], f32, tag=f"wraw{s}")
            wsum = smpool.tile([P, T], f32, tag=f"wsum{s}")
            wnrm = smpool.tile([P, T], f32, tag=f"wnrm{s}")
            wgt = smpool.tile([P, T * KNN], f32, tag=f"wgt{s}")
            nchunk = (S + 511) // 512
            BLK = 8
            for b0 in range(0, T, BLK):
              bn = min(BLK, T - b0)
              for t in range(b0, b0 + bn):
                nd_sb = ndpool.tile([P, S], f32, tag="nd_sb")
                pnd = ps_nd.tile([P, min(S, 1024)], f32, tag="pnd")
                for c in range(nchunk):
                    w = min(512, S - c * 512)
                    nc.tensor.matmul(
                        pnd[:, (c % 2) * 512:(c % 2) * 512 + w],
                        a5[:, t * P:(t + 1) * P],
                        b5[:, c * 512:c * 512 + w],
                        start=True, stop=True)
                    if c % 2 == 1 or c == nchunk - 1:
                        lo = (c // 2) * 1024
                        w2 = min(1024, S - lo)
                        nc.scalar.copy(nd_sb[:, lo:lo + w2], pnd[:, :w2])
                        if c != nchunk - 1:
                            pnd = ps_nd.tile([P, min(S, 1024)], f32, tag="pnd")
                nc.vector.max(maxb[:, t * 8:(t + 1) * 8], nd_sb[:])
                nc.vector.max_index(idxb[:, t * 8:(t + 1) * 8],
                                    maxb[:, t * 8:(t + 1) * 8], nd_sb[:])

              # weights for this block
              top3 = maxb[:, b0 * 8:(b0 + bn) * 8].rearrange(
                  "p (t e) -> p t e", e=8)[:, :, 0:KNN]
              d3 = dbuf[:, b0 * KNN:(b0 + bn) * KNN]
              w3 = wraw[:, b0 * KNN:(b0 + bn) * KNN]
              g3 = wgt[:, b0 * KNN:(b0 + bn) * KNN]
              nc.vector.tensor_scalar(d3.rearrange("p (t e) -> p t e", e=KNN),
                                      top3, -1.0, EPS, op0=Alu.mult, op1=Alu.add)
              nc.vector.reciprocal(w3, d3)
              nc.vector.tensor_reduce(
                  wsum[:, b0:b0 + bn], w3.rearrange("p (t e) -> p t e", e=KNN),
                  axis=Axis.X, op=Alu.add)
              nc.vector.reciprocal(wnrm[:, b0:b0 + bn], wsum[:, b0:b0 + bn])
              nc.vector.tensor_tensor(
                  g3.rearrange("p (t e) -> p t e", e=KNN),
                  w3.rearrange("p (t e) -> p t e", e=KNN),
                  wnrm[:, b0:b0 + bn].rearrange(
                      "p (t o) -> p t o", o=1).to_broadcast([P, bn, KNN]),
                  op=Alu.mult)

              # gather + weighted combine for this block
              for t in range(b0, b0 + bn):
                gts = []
                for k in range(KNN):
                    gt = gpool.tile([P, D2], f32, tag=f"g{k}")
                    nc.gpsimd.indirect_dma_start(
                        out=gt[:], out_offset=None,
                        in_=tsrc[s][:, :],
                        in_offset=IndirectOffsetOnAxis(
                            ap=idxb[:, t * 8 + k:t * 8 + k + 1], axis=0))
                    gts.append(gt)
                ra = rpool.tile([P, D2], f32, tag="ra")
                rb = rpool.tile([P, D2], f32, tag="rb")
                nc.scalar.activation(ra[:], gts[0][:],
                                     mybir.ActivationFunctionType.Identity,
                                     scale=wgt[:, t * 3:t * 3 + 1])
                nc.vector.scalar_tensor_tensor(
                    rb[:], gts[1][:], wgt[:, t * 3 + 1:t * 3 + 2], ra[:],
                    op0=Alu.mult, op1=Alu.add)
                nc.vector.scalar_tensor_tensor(
                    ra[:], gts[2][:], wgt[:, t * 3 + 2:t * 3 + 3], rb[:],
                    op0=Alu.mult, op1=Alu.add)

                if s < 3:
                    nc.sync.dma_start(
                        tsrc[s + 1][t * P:(t + 1) * P, st["cx"]:st["cx"] + D2],
                        ra[:])
                else:
                    # transpose [128, D2] to column layout and DMA to oi
                    dchunks = D2 // P
                    colb = rpool.tile([P, D2], f32, tag="colb")
                    for dd in range(dchunks):
                        pst = ps_tp.tile([P, P], f32, tag="pst")
                        nc.tensor.transpose(
                            pst[:], ra[:, dd * P:(dd + 1) * P], ident[:])
                        nc.scalar.copy(colb[:, dd * P:(dd + 1) * P], pst[:])
                    nc.sync.dma_start(
                        oi.rearrange("(dd p) (t j) -> p dd t j",
                                     p=P, j=P)[:, :, t, :],
                        colb[:].rearrange("p (dd j) -> p dd j", j=P))
    if split_waits:
        _split_multi_waits(nc)
    return nc


def _split_multi_waits(nc):
    """This walrus build rejects instructions carrying more than one sync
    wait. Hoist extra waits into same-engine NoOps inserted just before."""
    import concourse.mybir as mybir

    n = 0
    for f in nc.m.functions:
        for bb in f.blocks:
            il = bb.instructions
            i = 0
            while i < len(il):
                inst = il[i]
                si = getattr(inst, "sync_info", None)
                ow = list(si.on_wait) if si is not None else []
                if len(ow) > 1:
                    for w in ow[:-1]:
                        nop = mybir.InstNoOp(name=f"W{n}-{inst.name}",
                                             ins=[], outs=[])
                        n += 1
                        nop.engine = inst.engine
                        nop.sync_info = mybir.SyncInfo(on_update=[],
                                                       on_wait=[w])
                        il.insert(i, nop)
                        i += 1
                    inst.sync_info = mybir.SyncInfo(
                        on_update=list(si.on_update), on_wait=[ow[-1]])
                i += 1


def _patch_tile_drain():
    """This walrus build rejects >1 sync-wait on the kernel-tail Drain; spread
    the waits across single-wait SP nops instead."""
    import concourse.mybir as mybir
    import concourse.tile as tile
    from concourse.vector_clock import ScopedClock

    if getattr(tile.TileContext, "_drain_patched", False):
        return

    def _patched(self, tick_clock, wait_clock):
        nc = self.nc
        probe = nc.sync.nop()
        wait_clock.add_sem_waits(probe.ins,
                                 ScopedClock({None: tick_clock.global_clock}))
        si = probe.ins.sync_info
        ow = list(si.on_wait) if si is not None else []
        if len(ow) > 1:
            for w in ow[1:]:
                n2 = nc.sync.nop()
                n2.ins.sync_info = mybir.SyncInfo(on_update=[], on_wait=[w])
            probe.ins.sync_info = mybir.SyncInfo(on_update=list(si.on_update),
                                                 on_wait=[ow[0]])
        nc.sync.drain()
        nc.all_engine_barrier()
        assert self.sems is not None
        popped = nc._tile_sem_poison_stack.pop()
        assert popped is self._sem_poison
        nc.clear_and_free_semaphores(list(self.sems.allocated().values()))
        nc.all_engine_barrier()

    tile.TileContext._drain_and_barrier = _patched
    tile.TileContext._drain_patched = True


def _get_program(ns, cs, n_half):
    key = (tuple(ns), tuple(cs), n_half)
    if _CACHED["key"] != key:
        _CACHED["nc"] = _build_program(ns, cs, n_half)
        _CACHED["key"] = key
    return _CACHED["nc"]


def make_core_inputs(inputs, ns, n_half, core):
    """Slice/transform full inputs for one core (b = core//2, half = core%2)."""
    b, h = core // 2, core % 2
    d = {}
    x0h = np.ascontiguousarray(np.asarray(inputs["xyz0"])[b, h * n_half:(h + 1) * n_half])
    d["xt0"] = np.ascontiguousarray(x0h.T)
    d["xr0"] = x0h
    for i in range(1, 5):
        xi = np.ascontiguousarray(np.asarray(inputs[f"xyz{i}"])[b])
        d[f"xt{i}"] = np.ascontiguousarray(xi.T)
        d[f"xr{i}"] = xi
        d[f"f{i}"] = np.ascontiguousarray(np.asarray(inputs[f"x{i}"])[b])
    return d


def kernel(**inputs):
    from concourse.bass_utils import run_bass_kernel_spmd

    ns, cs = NS, CS
    n_half = ns[0] // 2
    nc = _get_program(ns, cs, n_half)

    in_maps = [make_core_inputs(inputs, ns, n_half, c) for c in range(8)]
    res = run_bass_kernel_spmd(nc, in_maps, core_ids=list(range(8)))

    dout = sum(cs)
    out = np.empty((B, dout, ns[0]), np.float32)
    out[:, :cs[0], :] = np.asarray(inputs["x0"])
    for c in range(8):
        b, h = c // 2, c % 2
        out[b, cs[0]:, h * n_half:(h + 1) * n_half] = res.results[c]["oi"]
    return out



# revision 5
# speedup vs baseline: 1.1824x; 1.1824x over previous
"""Trainium2 Bass kernel for a PointNet++-style feature-propagation decoder
(4 stages of kNN(k=3) inverse-distance-weighted feature interpolation).

Sharding: batch b = core//2 (data parallel over B=4); the finest stage's 8192
query points split in half across each core pair (point parallel along N per
the sharding hint). Stages 0-2 are duplicated within a pair; stage 3 is
n-split. Output rows 0:64 are the x0 passthrough, assembled on the host.

Per-core device pipeline per stage (all feature traffic in fp16):
  - negated partial distances nd = 2a.b - |b|^2 via one K=4 fp32 PE matmul
    per 128-query tile (the -|a|^2 term is constant per query and cannot
    change each query's top-k order; it is restored only for the 3 selected
    values when computing weights). |b|^2 rows come from a tiny ones-matmul.
  - top-3 neighbors via DVE max (top-8) + max_index read directly off PSUM
  - inverse-distance weights on DVE (batched per 8-tile block)
  - feature gather via SWDGE indirect DMA from fp16 DRAM tables
  - weighted 3-way combine on the PE: psum += diag(w_k) @ gathered_k, where
    diag(w_k) = identity * w_k built by one 4x-mode DVE tensor_scalar each
  - stage results are written as fp16 table rows for the next stage's gather;
    the final stage writes fp16 [n, d] rows that the host transposes.
"""

import numpy as np

P = 128
KNN = 3
EPS = 1e-8
BLK = 8

B = 4
NS = [8192, 2048, 512, 128, 32]  # points per level, finest -> coarsest
CS = [64, 128, 256, 512, 1024]   # feature channels per level

_CACHED = {"nc": None, "key": None}


def _build_program(ns, cs, n_half, split_waits=True):
    """Trace the per-core Bass program. ns/cs as in reference (finest first).
    n_half: number of finest-level query points this core handles."""
    import contextlib

    import concourse.bass as bass
    import concourse.mybir as mybir
    import concourse.tile as tile
    from concourse.bass import IndirectOffsetOnAxis
    from concourse.masks import make_identity

    _patch_tile_drain()

    f32 = mybir.dt.float32
    f16 = mybir.dt.float16
    u32 = mybir.dt.uint32
    Alu = mybir.AluOpType
    Axis = mybir.AxisListType

    # stage s: fine level (3-s) [this core's half of level 0 for s=3],
    # coarse level (4-s). d2[s]: width of the table gathered at stage s.
    d2 = [cs[4]]
    for s in range(1, 4):
        d2.append(cs[4 - s] + d2[s - 1])
    stages = []
    for s in range(4):
        nf = ns[3 - s] if s < 3 else n_half
        stages.append(dict(nf=nf, S=ns[4 - s], d2=d2[s],
                           cx=(cs[3 - s] if s < 3 else None)))

    nc = bass.Bass("TRN2")

    # ---- external inputs (per core) ----
    xt = {}   # xyz transposed [3, N] f32
    xr = {}   # xyz rows [N, 3] f32
    for i, n in enumerate(ns):
        nn_ = n_half if i == 0 else n
        xt[i] = nc.dram_tensor(f"xt{i}", [3, nn_], f32, kind="ExternalInput")
        xr[i] = nc.dram_tensor(f"xr{i}", [nn_, 3], f32, kind="ExternalInput")
    ft = {}   # features transposed [N, C] fp16
    for i in range(1, 5):
        ft[i] = nc.dram_tensor(f"ft{i}", [ns[i], cs[i]], f16,
                               kind="ExternalInput")

    # ---- external output: interp part of final stage, fp16 rows ----
    oi = nc.dram_tensor("oi", [n_half, d2[3]], f16, kind="ExternalOutput")

    # ---- internal fp16 DRAM source tables (stage 0 gathers ft[4]) ----
    tsrc = {s: nc.dram_tensor(f"tsrc{s}", [stages[s]["S"], stages[s]["d2"]],
                              f16)
            for s in range(1, 4)}
    gsrc = {0: ft[4], 1: tsrc[1], 2: tsrc[2], 3: tsrc[3]}

    with tile.TileContext(nc) as tc, contextlib.ExitStack() as ctx:
        cpool = ctx.enter_context(tc.tile_pool(name="const", bufs=1))
        abpool = ctx.enter_context(tc.tile_pool(name="ab", bufs=2))
        smpool = ctx.enter_context(tc.tile_pool(name="sm", bufs=2))
        gpool = ctx.enter_context(tc.tile_pool(name="g", bufs=3))
        dpool = ctx.enter_context(tc.tile_pool(name="d", bufs=3))
        rpool = ctx.enter_context(tc.tile_pool(name="r", bufs=3))
        ps_nd = ctx.enter_context(tc.tile_pool(name="ps_nd", bufs=1,
                                               space="PSUM"))
        ps_cb = ctx.enter_context(tc.tile_pool(name="ps_cb", bufs=2,
                                               space="PSUM"))

        ident = cpool.tile([P, P], f16, tag="ident")
        make_identity(nc, ident[:])
        maxn = max(st["nf"] for st in stages)
        ones = cpool.tile([1, maxn], f32, tag="ones")
        nc.gpsimd.memset(ones[:], 1.0)
        neg3 = cpool.tile([3, 1], f32, tag="neg3")
        nc.gpsimd.memset(neg3[:], -1.0)

        # ---- x^T parts of the source tables (DRAM->DRAM) ----
        nc.sync.dma_start(tsrc[1][:, 0:cs[3]], ft[3][:, :])
        nc.sync.dma_start(tsrc[2][:, 0:cs[2]], ft[2][:, :])
        nc.sync.dma_start(tsrc[3][:, 0:cs[1]], ft[1][:, :])

        # ---------- stages ----------
        for s, st in enumerate(stages):
            nf, S, D2 = st["nf"], st["S"], st["d2"]
            T = nf // P
            fine = 3 - s if s < 3 else 0
            coarse = 4 - s
            nchunk = (S + 511) // 512

            # B-side [4, S]: rows 0-2 = 2*xt_coarse, row 3 = -|b|^2
            b4 = abpool.tile([4, S], f32, tag=f"b4{s}")
            nc.sync.dma_start(b4[0:3, :], xt[coarse][:, :])
            sqb = abpool.tile([3, S], f32, tag=f"sqb{s}")
            nc.vector.tensor_tensor(sqb[:], b4[0:3, :], b4[0:3, :],
                                    op=Alu.mult)
            nc.scalar.mul(b4[0:3, :], b4[0:3, :], 2.0)
            nrm = abpool.tile([1, S], f32, tag=f"nrm{s}")
            for h0 in range(0, S, 1024):
                hw_ = min(1024, S - h0)
                pnb = ps_cb.tile([P, 1024], f32, tag="pcb")
                for c0 in range(0, hw_, 512):
                    w = min(512, hw_ - c0)
                    nc.tensor.matmul(pnb[0:1, c0:c0 + w], neg3[:],
                                     sqb[:, h0 + c0:h0 + c0 + w],
                                     start=True, stop=True)
                nc.scalar.copy(nrm[0:1, h0:h0 + hw_], pnb[0:1, :hw_])
            nc.sync.dma_start(b4[3:4, :], nrm[:])

            # A-side [4, nf]: rows 0-2 = xt_fine, row 3 = 1
            a4 = abpool.tile([4, nf], f32, tag=f"a4{s}")
            nc.sync.dma_start(a4[0:3, :], xt[fine][:, :])
            nc.sync.dma_start(a4[3:4, :], ones[:, :nf])

            # query norms + eps, row layout: point t*128+p at [p, t]
            rx = smpool.tile([P, T * 3], f32, tag=f"rx{s}")
            nc.sync.dma_start(
                rx[:].rearrange("p (t c) -> p t c", c=3),
                xr[fine][:, :].rearrange("(t p) c -> p t c", p=P))
            sqa = smpool.tile([P, T * 3], f32, tag=f"sqa{s}")
            nc.vector.tensor_tensor(sqa[:], rx[:], rx[:], op=Alu.mult)
            anrm = smpool.tile([P, T], f32, tag=f"anrm{s}")
            nc.vector.tensor_reduce(
                anrm[:], sqa[:].rearrange("p (t c) -> p t c", c=3),
                axis=Axis.X, op=Alu.add)

            # per-stage arrays
            maxb = smpool.tile([P, T * 8], f32, tag=f"maxb{s}")
            idxb = smpool.tile([P, T * 8], u32, tag=f"idxb{s}")
            d3 = smpool.tile([P, T * KNN], f32, tag=f"d3{s}")
            w3 = smpool.tile([P, T * KNN], f32, tag=f"w3{s}")
            wgt = smpool.tile([P, T * KNN], f32, tag=f"wgt{s}")
            wsum = smpool.tile([P, T], f32, tag=f"wsum{s}")

            for b0 in range(0, T, BLK):
                bn = min(BLK, T - b0)
                # distances + top-3 (values+indices) per tile
                for t in range(b0, b0 + bn):
                    pnd = ps_nd.tile([P, 2048], f32, tag="pnd")
                    for c in range(nchunk):
                        w = min(512, S - c * 512)
                        nc.tensor.matmul(
                            pnd[:, c * 512:c * 512 + w],
                            a4[:, t * P:(t + 1) * P],
                            b4[:, c * 512:c * 512 + w],
                            start=True, stop=True)
                    nc.vector.max(maxb[:, t * 8:(t + 1) * 8], pnd[:, :S])
                    nc.vector.max_index(idxb[:, t * 8:(t + 1) * 8],
                                        maxb[:, t * 8:(t + 1) * 8],
                                        pnd[:, :S])

                # block weights: d = (|a|^2+eps) - nd, w = 1/d normalized
                top3 = maxb[:, b0 * 8:(b0 + bn) * 8].rearrange(
                    "p (t e) -> p t e", e=8)[:, :, 0:KNN]
                d3v = d3[:, b0 * KNN:(b0 + bn) * KNN]
                w3v = w3[:, b0 * KNN:(b0 + bn) * KNN]
                gv = wgt[:, b0 * KNN:(b0 + bn) * KNN]
                nc.vector.scalar_tensor_tensor(
                    d3v.rearrange("p (t e) -> p t e", e=KNN), top3, -1.0,
                    anrm[:, b0:b0 + bn].rearrange(
                        "p (t o) -> p t o", o=1).to_broadcast([P, bn, KNN]),
                    op0=Alu.mult, op1=Alu.add)
                nc.vector.tensor_scalar_add(d3v, d3v, EPS)
                nc.vector.reciprocal(w3v, d3v)
                nc.vector.tensor_reduce(
                    wsum[:, b0:b0 + bn],
                    w3v.rearrange("p (t e) -> p t e", e=KNN),
                    axis=Axis.X, op=Alu.add)
                nc.vector.reciprocal(wsum[:, b0:b0 + bn],
                                     wsum[:, b0:b0 + bn])
                nc.vector.tensor_tensor(
                    gv.rearrange("p (t e) -> p t e", e=KNN),
                    w3v.rearrange("p (t e) -> p t e", e=KNN),
                    wsum[:, b0:b0 + bn].rearrange(
                        "p (t o) -> p t o", o=1).to_broadcast([P, bn, KNN]),
                    op=Alu.mult)

                # gather + PE weighted combine per tile
                for t in range(b0, b0 + bn):
                    gt = gpool.tile([P, KNN * D2], f16, tag="gt")
                    for k in range(KNN):
                        nc.gpsimd.indirect_dma_start(
                            out=gt[:, k * D2:(k + 1) * D2], out_offset=None,
                            in_=gsrc[s][:, :],
                            in_offset=IndirectOffsetOnAxis(
                                ap=idxb[:, t * 8 + k:t * 8 + k + 1], axis=0))
                    diag = dpool.tile([P, KNN * P], f16, tag="diag")
                    for k in range(KNN):
                        nc.vector.tensor_scalar_mul(
                            diag[:, k * P:(k + 1) * P], ident[:],
                            wgt[:, t * KNN + k:t * KNN + k + 1])
                    res = rpool.tile([P, D2], f16, tag="res")
                    for h0 in range(0, D2, 1024):
                        hw_ = min(1024, D2 - h0)
                        pcb = ps_cb.tile([P, 1024], f32, tag="pcb")
                        for c0 in range(0, hw_, 512):
                            w = min(512, hw_ - c0)
                            for k in range(KNN):
                                nc.tensor.matmul(
                                    pcb[:, c0:c0 + w],
                                    diag[:, k * P:(k + 1) * P],
                                    gt[:, k * D2 + h0 + c0:
                                       k * D2 + h0 + c0 + w],
                                    start=(k == 0), stop=(k == KNN - 1))
                        nc.scalar.copy(res[:, h0:h0 + hw_], pcb[:, :hw_])
                    if s < 3:
                        nc.sync.dma_start(
                            tsrc[s + 1][t * P:(t + 1) * P,
                                        st["cx"]:st["cx"] + D2], res[:])
                    else:
                        nc.sync.dma_start(oi[t * P:(t + 1) * P, :], res[:])
    if split_waits:
        _split_multi_waits(nc)
    return nc


def _split_multi_waits(nc):
    """This walrus build rejects instructions carrying more than one sync
    wait. Hoist extra waits into same-engine NoOps inserted just before."""
    import concourse.mybir as mybir

    n = 0
    for f in nc.m.functions:
        for bb in f.blocks:
            il = bb.instructions
            i = 0
            while i < len(il):
                inst = il[i]
                si = getattr(inst, "sync_info", None)
                ow = list(si.on_wait) if si is not None else []
                if len(ow) > 1:
                    for w in ow[:-1]:
                        nop = mybir.InstNoOp(name=f"W{n}-{inst.name}",
                                             ins=[], outs=[])
                        n += 1
                        nop.engine = inst.engine
                        nop.sync_info = mybir.SyncInfo(on_update=[],
                                                       on_wait=[w])
                        il.insert(i, nop)
                        i += 1
                    inst.sync_info = mybir.SyncInfo(
                        on_update=list(si.on_update), on_wait=[ow[-1]])
                i += 1


def _patch_tile_drain():
    """This walrus build rejects >1 sync-wait on the kernel-tail Drain; spread
    the waits across single-wait SP nops instead."""
    import concourse.mybir as mybir
    import concourse.tile as tile
    from concourse.vector_clock import ScopedClock

    if getattr(tile.TileContext, "_drain_patched", False):
        return

    def _patched(self, tick_clock, wait_clock):
        nc = self.nc
        probe = nc.sync.nop()
        wait_clock.add_sem_waits(probe.ins,
                                 ScopedClock({None: tick_clock.global_clock}))
        si = probe.ins.sync_info
        ow = list(si.on_wait) if si is not None else []
        if len(ow) > 1:
            for w in ow[1:]:
                n2 = nc.sync.nop()
                n2.ins.sync_info = mybir.SyncInfo(on_update=[], on_wait=[w])
            probe.ins.sync_info = mybir.SyncInfo(on_update=list(si.on_update),
                                                 on_wait=[ow[0]])
        nc.sync.drain()
        nc.all_engine_barrier()
        assert self.sems is not None
        popped = nc._tile_sem_poison_stack.pop()
        assert popped is self._sem_poison
        nc.clear_and_free_semaphores(list(self.sems.allocated().values()))
        nc.all_engine_barrier()

    tile.TileContext._drain_and_barrier = _patched
    tile.TileContext._drain_patched = True


def _get_program(ns, cs, n_half):
    key = (tuple(ns), tuple(cs), n_half)
    if _CACHED["key"] != key:
        _CACHED["nc"] = _build_program(ns, cs, n_half)
        _CACHED["key"] = key
    return _CACHED["nc"]


def make_core_inputs(inputs, ns, n_half, core):
    """Slice/transform full inputs for one core (b = core//2, half = core%2)."""
    b, h = core // 2, core % 2
    d = {}
    x0h = np.ascontiguousarray(
        np.asarray(inputs["xyz0"])[b, h * n_half:(h + 1) * n_half])
    d["xt0"] = np.ascontiguousarray(x0h.T)
    d["xr0"] = x0h
    for i in range(1, 5):
        xi = np.ascontiguousarray(np.asarray(inputs[f"xyz{i}"])[b])
        d[f"xt{i}"] = np.ascontiguousarray(xi.T)
        d[f"xr{i}"] = xi
        d[f"ft{i}"] = np.ascontiguousarray(
            np.asarray(inputs[f"x{i}"])[b].T.astype(np.float16))
    return d


def kernel(**inputs):
    from concourse.bass_utils import run_bass_kernel_spmd

    ns, cs = NS, CS
    n_half = ns[0] // 2
    nc = _get_program(ns, cs, n_half)

    in_maps = [make_core_inputs(inputs, ns, n_half, c) for c in range(8)]
    res = run_bass_kernel_spmd(nc, in_maps, core_ids=list(range(8)))

    dout = sum(cs)
    out = np.empty((B, dout, ns[0]), np.float32)
    out[:, :cs[0], :] = np.asarray(inputs["x0"])
    for c in range(8):
        b, h = c // 2, c % 2
        out[b, cs[0]:, h * n_half:(h + 1) * n_half] = \
            res.results[c]["oi"].astype(np.float32).T
    return out


# revision 8
# speedup vs baseline: 1.3952x; 1.1800x over previous
"""Trainium2 Bass kernel for a PointNet++-style feature-propagation decoder
(4 stages of kNN(k=3) inverse-distance-weighted feature interpolation).

Sharding: batch b = core//2 (data parallel over B=4); the finest stage's 8192
query points split in half across each core pair (point parallel along N per
the sharding hint). Stages 0-2 are duplicated within a pair; stage 3 is
n-split. Output rows 0:64 are the x0 passthrough, assembled on the host.

Per-core device pipeline per stage (all feature traffic in fp16):
  - negated partial distances nd = 2a.b - |b|^2 via one K=4 fp32 PE matmul
    per 128-query tile (the -|a|^2 term is constant per query and cannot
    change each query's top-k order; it is restored only for the 3 selected
    values when computing weights). |b|^2 rows come from a tiny ones-matmul.
  - top-3 neighbors via DVE max (top-8) + max_index read directly off PSUM
  - inverse-distance weights on DVE (batched per 8-tile block)
  - feature gather via SWDGE indirect DMA from fp16 DRAM tables
  - weighted 3-way combine on the PE: psum += diag(w_k) @ gathered_k, where
    diag(w_k) = identity * w_k built by one 4x-mode DVE tensor_scalar each
  - stage results are written as fp16 table rows for the next stage's gather;
    the final stage writes fp16 [n, d] rows that the host transposes.
"""

import numpy as np

P = 128
KNN = 3
EPS = 1e-8
LAG = 2

B = 4
NS = [8192, 2048, 512, 128, 32]  # points per level, finest -> coarsest
CS = [64, 128, 256, 512, 1024]   # feature channels per level

_CACHED = {"nc": None, "key": None}


def _build_program(ns, cs, n_half, split_waits=True):
    """Trace the per-core Bass program. ns/cs as in reference (finest first).
    n_half: number of finest-level query points this core handles."""
    import contextlib

    import concourse.bass as bass
    import concourse.mybir as mybir
    import concourse.tile as tile
    from concourse.bass import IndirectOffsetOnAxis
    from concourse.masks import make_identity

    _patch_tile_drain()

    f32 = mybir.dt.float32
    f16 = mybir.dt.float16
    u32 = mybir.dt.uint32
    Alu = mybir.AluOpType
    Axis = mybir.AxisListType

    # stage s: fine level (3-s) [this core's half of level 0 for s=3],
    # coarse level (4-s). d2[s]: width of the table gathered at stage s.
    d2 = [cs[4]]
    for s in range(1, 4):
        d2.append(cs[4 - s] + d2[s - 1])
    stages = []
    for s in range(4):
        nf = ns[3 - s] if s < 3 else n_half
        stages.append(dict(nf=nf, S=ns[4 - s], d2=d2[s],
                           cx=(cs[3 - s] if s < 3 else None)))

    nc = bass.Bass("TRN2")

    # ---- external inputs (per core) ----
    xt = {}   # xyz transposed [3, N] f32
    xr = {}   # xyz rows [N, 3] f32
    for i, n in enumerate(ns):
        nn_ = n_half if i == 0 else n
        xt[i] = nc.dram_tensor(f"xt{i}", [3, nn_], f32, kind="ExternalInput")
        xr[i] = nc.dram_tensor(f"xr{i}", [nn_, 3], f32, kind="ExternalInput")
    ft = {}   # features transposed [N, C] fp16
    for i in range(1, 5):
        ft[i] = nc.dram_tensor(f"ft{i}", [ns[i], cs[i]], f16,
                               kind="ExternalInput")

    # ---- external output: interp part of final stage, fp16 rows ----
    oi = nc.dram_tensor("oi", [n_half, d2[3]], f16, kind="ExternalOutput")

    # ---- internal fp16 DRAM source tables (stage 0 gathers ft[4]) ----
    tsrc = {s: nc.dram_tensor(f"tsrc{s}", [stages[s]["S"], stages[s]["d2"]],
                              f16)
            for s in range(1, 4)}
    gsrc = {0: ft[4], 1: tsrc[1], 2: tsrc[2], 3: tsrc[3]}

    with tile.TileContext(nc) as tc, contextlib.ExitStack() as ctx:
        cpool = ctx.enter_context(tc.tile_pool(name="const", bufs=1))
        abpool = ctx.enter_context(tc.tile_pool(name="ab", bufs=2))
        smpool = ctx.enter_context(tc.tile_pool(name="sm", bufs=2))
        gpool = ctx.enter_context(tc.tile_pool(name="g", bufs=4))
        dpool = ctx.enter_context(tc.tile_pool(name="d", bufs=4))
        rpool = ctx.enter_context(tc.tile_pool(name="r", bufs=3))
        ps_nd = ctx.enter_context(tc.tile_pool(name="ps_nd", bufs=1,
                                               space="PSUM"))
        ps_cb = ctx.enter_context(tc.tile_pool(name="ps_cb", bufs=2,
                                               space="PSUM"))

        ident = cpool.tile([P, P], f16, tag="ident")
        make_identity(nc, ident[:])
        maxn = max(st["nf"] for st in stages)
        ones = cpool.tile([1, maxn], f32, tag="ones")
        nc.gpsimd.memset(ones[:], 1.0)
        neg3 = cpool.tile([3, 1], f32, tag="neg3")
        nc.gpsimd.memset(neg3[:], -1.0)

        # ---- x^T parts of the source tables (DRAM->DRAM) ----
        nc.sync.dma_start(tsrc[1][:, 0:cs[3]], ft[3][:, :])
        nc.sync.dma_start(tsrc[2][:, 0:cs[2]], ft[2][:, :])
        nc.sync.dma_start(tsrc[3][:, 0:cs[1]], ft[1][:, :])

        # ---------- stages ----------
        for s, st in enumerate(stages):
            nf, S, D2 = st["nf"], st["S"], st["d2"]
            T = nf // P
            fine = 3 - s if s < 3 else 0
            coarse = 4 - s
            nchunk = (S + 511) // 512

            # B-side [4, S]: rows 0-2 = 2*xt_coarse, row 3 = -|b|^2
            b4 = abpool.tile([4, S], f32, tag=f"b4{s}")
            nc.sync.dma_start(b4[0:3, :], xt[coarse][:, :])
            sqb = abpool.tile([3, S], f32, tag=f"sqb{s}")
            nc.vector.tensor_tensor(sqb[:], b4[0:3, :], b4[0:3, :],
                                    op=Alu.mult)
            nc.scalar.mul(b4[0:3, :], b4[0:3, :], 2.0)
            nrm = abpool.tile([1, S], f32, tag=f"nrm{s}")
            for h0 in range(0, S, 1024):
                hw_ = min(1024, S - h0)
                pnb = ps_cb.tile([P, 1024], f32, tag="pcb")
                for c0 in range(0, hw_, 512):
                    w = min(512, hw_ - c0)
                    nc.tensor.matmul(pnb[0:1, c0:c0 + w], neg3[:],
                                     sqb[:, h0 + c0:h0 + c0 + w],
                                     start=True, stop=True)
                nc.scalar.copy(nrm[0:1, h0:h0 + hw_], pnb[0:1, :hw_])
            nc.sync.dma_start(b4[3:4, :], nrm[:])

            # A-side [4, nf]: rows 0-2 = xt_fine, row 3 = 1
            a4 = abpool.tile([4, nf], f32, tag=f"a4{s}")
            nc.sync.dma_start(a4[0:3, :], xt[fine][:, :])
            nc.sync.dma_start(a4[3:4, :], ones[:, :nf])

            # query norms + eps, row layout: point t*128+p at [p, t]
            rx = smpool.tile([P, T * 3], f32, tag=f"rx{s}")
            nc.sync.dma_start(
                rx[:].rearrange("p (t c) -> p t c", c=3),
                xr[fine][:, :].rearrange("(t p) c -> p t c", p=P))
            sqa = smpool.tile([P, T * 3], f32, tag=f"sqa{s}")
            nc.vector.tensor_tensor(sqa[:], rx[:], rx[:], op=Alu.mult)
            anrm = smpool.tile([P, T], f32, tag=f"anrm{s}")
            nc.vector.tensor_reduce(
                anrm[:], sqa[:].rearrange("p (t c) -> p t c", c=3),
                axis=Axis.X, op=Alu.add)

            # per-stage arrays
            maxb = smpool.tile([P, T * 8], f32, tag=f"maxb{s}")
            idxb = smpool.tile([P, T * 8], u32, tag=f"idxb{s}")
            d3 = smpool.tile([P, T * KNN], f32, tag=f"d3{s}")
            w3 = smpool.tile([P, T * KNN], f32, tag=f"w3{s}")
            wgt = smpool.tile([P, T * KNN], f32, tag=f"wgt{s}")
            wsum = smpool.tile([P, T], f32, tag=f"wsum{s}")

            # software-pipelined tile loop: slot i runs the front half
            # (distances, top-3, gather launch, weights, diag) for tile i and
            # the back half (PE combine, psum copies, row write) for tile
            # i-LAG, so no engine's in-order stream sits behind work whose
            # inputs (the gathers) are still in flight.
            gts, diags = {}, {}
            for i in range(T + LAG):
                if i < T:
                    t = i
                    pnd = ps_nd.tile([P, 2048], f32, tag="pnd")
                    for c in range(nchunk):
                        w = min(512, S - c * 512)
                        nc.tensor.matmul(
                            pnd[:, c * 512:c * 512 + w],
                            a4[:, t * P:(t + 1) * P],
                            b4[:, c * 512:c * 512 + w],
                            start=True, stop=True)
                    nc.vector.max(maxb[:, t * 8:(t + 1) * 8], pnd[:, :S])
                    nc.vector.max_index(idxb[:, t * 8:(t + 1) * 8],
                                        maxb[:, t * 8:(t + 1) * 8],
                                        pnd[:, :S])
                    gt = gpool.tile([P, KNN * D2], f16, tag="gt")
                    gts[t] = gt
                    for k in range(KNN):
                        nc.gpsimd.indirect_dma_start(
                            out=gt[:, k * D2:(k + 1) * D2], out_offset=None,
                            in_=gsrc[s][:, :],
                            in_offset=IndirectOffsetOnAxis(
                                ap=idxb[:, t * 8 + k:t * 8 + k + 1], axis=0))
                    # per-tile weights: d = |a|^2 - nd + eps, w = 1/d, norm
                    t3 = slice(t * KNN, (t + 1) * KNN)
                    nc.vector.tensor_scalar(
                        d3[:, t3], maxb[:, t * 8:t * 8 + KNN], -1.0,
                        anrm[:, t:t + 1], op0=Alu.mult, op1=Alu.add)
                    nc.vector.tensor_scalar_add(d3[:, t3], d3[:, t3], EPS)
                    nc.vector.reciprocal(w3[:, t3], d3[:, t3])
                    nc.vector.tensor_reduce(
                        wsum[:, t:t + 1],
                        w3[:, t3].rearrange("p (o e) -> p o e", o=1),
                        axis=Axis.X, op=Alu.add)
                    nc.vector.reciprocal(wsum[:, t:t + 1], wsum[:, t:t + 1])
                    nc.vector.tensor_scalar_mul(wgt[:, t3], w3[:, t3],
                                                wsum[:, t:t + 1])
                    diag = dpool.tile([P, KNN * P], f16, tag="diag")
                    diags[t] = diag
                    for k in range(KNN):
                        nc.vector.tensor_scalar_mul(
                            diag[:, k * P:(k + 1) * P], ident[:],
                            wgt[:, t * KNN + k:t * KNN + k + 1])
                if i >= LAG:
                    t = i - LAG
                    gt, diag = gts.pop(t), diags.pop(t)
                    res = rpool.tile([P, D2], f16, tag="res")
                    for h0 in range(0, D2, 1024):
                        hw_ = min(1024, D2 - h0)
                        pcb = ps_cb.tile([P, 1024], f32, tag="pcb")
                        for c0 in range(0, hw_, 512):
                            w = min(512, hw_ - c0)
                            for k in range(KNN):
                                nc.tensor.matmul(
                                    pcb[:, c0:c0 + w],
                                    diag[:, k * P:(k + 1) * P],
                                    gt[:, k * D2 + h0 + c0:
                                       k * D2 + h0 + c0 + w],
                                    start=(k == 0), stop=(k == KNN - 1))
                        nc.scalar.copy(res[:, h0:h0 + hw_], pcb[:, :hw_])
                    if s < 3:
                        nc.sync.dma_start(
                            tsrc[s + 1][t * P:(t + 1) * P,
                                        st["cx"]:st["cx"] + D2], res[:])
                    else:
                        nc.sync.dma_start(oi[t * P:(t + 1) * P, :], res[:])
    if split_waits:
        _split_multi_waits(nc)
    return nc


def _split_multi_waits(nc):
    """This walrus build rejects instructions carrying more than one sync
    wait. Hoist extra waits into same-engine NoOps inserted just before."""
    import concourse.mybir as mybir

    n = 0
    for f in nc.m.functions:
        for bb in f.blocks:
            il = bb.instructions
            i = 0
            while i < len(il):
                inst = il[i]
                si = getattr(inst, "sync_info", None)
                ow = list(si.on_wait) if si is not None else []
                if len(ow) > 1:
                    for w in ow[:-1]:
                        nop = mybir.InstNoOp(name=f"W{n}-{inst.name}",
                                             ins=[], outs=[])
                        n += 1
                        nop.engine = inst.engine
                        nop.sync_info = mybir.SyncInfo(on_update=[],
                                                       on_wait=[w])
                        il.insert(i, nop)
                        i += 1
                    inst.sync_info = mybir.SyncInfo(
                        on_update=list(si.on_update), on_wait=[ow[-1]])
                i += 1


def _patch_tile_drain():
    """This walrus build rejects >1 sync-wait on the kernel-tail Drain; spread
    the waits across single-wait SP nops instead."""
    import concourse.mybir as mybir
    import concourse.tile as tile
    from concourse.vector_clock import ScopedClock

    if getattr(tile.TileContext, "_drain_patched", False):
        return

    def _patched(self, tick_clock, wait_clock):
        nc = self.nc
        probe = nc.sync.nop()
        wait_clock.add_sem_waits(probe.ins,
                                 ScopedClock({None: tick_clock.global_clock}))
        si = probe.ins.sync_info
        ow = list(si.on_wait) if si is not None else []
        if len(ow) > 1:
            for w in ow[1:]:
                n2 = nc.sync.nop()
                n2.ins.sync_info = mybir.SyncInfo(on_update=[], on_wait=[w])
            probe.ins.sync_info = mybir.SyncInfo(on_update=list(si.on_update),
                                                 on_wait=[ow[0]])
        nc.sync.drain()
        nc.all_engine_barrier()
        assert self.sems is not None
        popped = nc._tile_sem_poison_stack.pop()
        assert popped is self._sem_poison
        nc.clear_and_free_semaphores(list(self.sems.allocated().values()))
        nc.all_engine_barrier()

    tile.TileContext._drain_and_barrier = _patched
    tile.TileContext._drain_patched = True


def _get_program(ns, cs, n_half):
    key = (tuple(ns), tuple(cs), n_half)
    if _CACHED["key"] != key:
        _CACHED["nc"] = _build_program(ns, cs, n_half)
        _CACHED["key"] = key
    return _CACHED["nc"]


def make_core_inputs(inputs, ns, n_half, core):
    """Slice/transform full inputs for one core (b = core//2, half = core%2)."""
    b, h = core // 2, core % 2
    d = {}
    x0h = np.ascontiguousarray(
        np.asarray(inputs["xyz0"])[b, h * n_half:(h + 1) * n_half])
    d["xt0"] = np.ascontiguousarray(x0h.T)
    d["xr0"] = x0h
    for i in range(1, 5):
        xi = np.ascontiguousarray(np.asarray(inputs[f"xyz{i}"])[b])
        d[f"xt{i}"] = np.ascontiguousarray(xi.T)
        d[f"xr{i}"] = xi
        d[f"ft{i}"] = np.ascontiguousarray(
            np.asarray(inputs[f"x{i}"])[b].T.astype(np.float16))
    return d


def kernel(**inputs):
    from concourse.bass_utils import run_bass_kernel_spmd

    ns, cs = NS, CS
    n_half = ns[0] // 2
    nc = _get_program(ns, cs, n_half)

    in_maps = [make_core_inputs(inputs, ns, n_half, c) for c in range(8)]
    res = run_bass_kernel_spmd(nc, in_maps, core_ids=list(range(8)))

    dout = sum(cs)
    out = np.empty((B, dout, ns[0]), np.float32)
    out[:, :cs[0], :] = np.asarray(inputs["x0"])
    for c in range(8):
        b, h = c // 2, c % 2
        out[b, cs[0]:, h * n_half:(h + 1) * n_half] = \
            res.results[c]["oi"].astype(np.float32).T
    return out


# revision 15
# speedup vs baseline: 1.7598x; 1.2613x over previous
"""Trainium2 Bass kernel for a PointNet++-style feature-propagation decoder
(4 stages of kNN(k=3) inverse-distance-weighted feature interpolation).

Sharding: batch b = core//2 (data parallel over B=4); the finest stage's 8192
query points split in half across each core pair (point parallel along N per
the sharding hint). Stages 0-2 are duplicated within a pair; stage 3 is
n-split. Output rows 0:64 are the x0 passthrough, assembled on the host.

Per-core device pipeline per stage (all feature traffic in fp16):
  - negated partial distances nd = 2a.b - |b|^2 via one K=4 fp32 PE matmul
    per 128-query tile (the -|a|^2 term is constant per query and cannot
    change each query's top-k order; it is restored only for the 3 selected
    values when computing weights). |b|^2 rows come from a tiny ones-matmul.
  - top-3 neighbors via DVE max (top-8) + max_index read directly off PSUM
  - inverse-distance weights on DVE (batched per 8-tile block)
  - feature gather via SWDGE indirect DMA from fp16 DRAM tables
  - weighted 3-way combine on the PE: psum += diag(w_k) @ gathered_k, where
    diag(w_k) = identity * w_k built by one 4x-mode DVE tensor_scalar each
  - stage results are written as fp16 table rows for the next stage's gather;
    the final stage writes fp16 [n, d] rows that the host transposes.
"""

import numpy as np

P = 128
KNN = 3
EPS = 1e-8
LAG = 2

B = 4
NS = [8192, 2048, 512, 128, 32]  # points per level, finest -> coarsest
CS = [64, 128, 256, 512, 1024]   # feature channels per level

_CACHED = {"nc": None, "key": None}


def _build_program(ns, cs, n_half, split_waits=True):
    """Trace the per-core Bass program. ns/cs as in reference (finest first).
    n_half: number of finest-level query points this core handles."""
    import contextlib

    import concourse.bass as bass
    import concourse.mybir as mybir
    import concourse.tile as tile
    from concourse.bass import IndirectOffsetOnAxis
    from concourse.masks import make_identity

    _patch_tile_drain()

    f32 = mybir.dt.float32
    f16 = mybir.dt.float16
    u32 = mybir.dt.uint32
    Alu = mybir.AluOpType
    Axis = mybir.AxisListType

    # stage s: fine level (3-s) [this core's half of level 0 for s=3],
    # coarse level (4-s). d2[s]: width of the table gathered at stage s.
    d2 = [cs[4]]
    for s in range(1, 4):
        d2.append(cs[4 - s] + d2[s - 1])
    stages = []
    for s in range(4):
        nf = ns[3 - s] if s < 3 else n_half
        stages.append(dict(nf=nf, S=ns[4 - s], d2=d2[s],
                           cx=(cs[3 - s] if s < 3 else None)))

    nc = bass.Bass("TRN2")

    # ---- external inputs (per core) ----
    xt = {}   # xyz transposed [3, N] f32
    xr = {}   # xyz rows [N, 3] f32
    for i, n in enumerate(ns):
        nn_ = n_half if i == 0 else n
        xt[i] = nc.dram_tensor(f"xt{i}", [3, nn_], f32, kind="ExternalInput")
        xr[i] = nc.dram_tensor(f"xr{i}", [nn_, 3], f32, kind="ExternalInput")
    ft = {}   # features transposed [N, C] fp16
    for i in range(1, 5):
        ft[i] = nc.dram_tensor(f"ft{i}", [ns[i], cs[i]], f16,
                               kind="ExternalInput")

    onesd = nc.dram_tensor("ones", [1, max(n_half, ns[1])], f32,
                           kind="ExternalInput")

    # ---- external output: interp part of final stage, fp16 rows ----
    oi = nc.dram_tensor("oi", [n_half, d2[3]], f16, kind="ExternalOutput")

    # ---- internal fp16 DRAM source tables (stage 0 gathers ft[4]) ----
    tsrc = {s: nc.dram_tensor(f"tsrc{s}", [stages[s]["S"], stages[s]["d2"]],
                              f16)
            for s in range(1, 4)}
    gsrc = {0: ft[4], 1: tsrc[1], 2: tsrc[2], 3: tsrc[3]}

    with tile.TileContext(nc) as tc, contextlib.ExitStack() as ctx:
        cpool = ctx.enter_context(tc.tile_pool(name="const", bufs=1))
        abpool = ctx.enter_context(tc.tile_pool(name="ab", bufs=2))
        smpool = ctx.enter_context(tc.tile_pool(name="sm", bufs=2))
        gpool = ctx.enter_context(tc.tile_pool(name="g", bufs=4))
        dpool = ctx.enter_context(tc.tile_pool(name="d", bufs=4))
        rpool = ctx.enter_context(tc.tile_pool(name="r", bufs=3))
        ndpool = ctx.enter_context(tc.tile_pool(name="nd", bufs=2))
        ps_d = ctx.enter_context(tc.tile_pool(name="ps_d", bufs=2,
                                              space="PSUM"))
        ps_cb = ctx.enter_context(tc.tile_pool(name="ps_cb", bufs=2,
                                               space="PSUM"))

        ident = cpool.tile([P, P], f16, tag="ident")
        make_identity(nc, ident[:])
        neg3 = cpool.tile([3, 1], f32, tag="neg3")
        nc.gpsimd.memset(neg3[:], -1.0)

        # ---- x^T parts of the source tables (DRAM->DRAM) ----
        nc.sync.dma_start(tsrc[1][:, 0:cs[3]], ft[3][:, :])
        nc.sync.dma_start(tsrc[2][:, 0:cs[2]], ft[2][:, :])
        nc.sync.dma_start(tsrc[3][:, 0:cs[1]], ft[1][:, :])

        # ---------- stages ----------
        for s, st in enumerate(stages):
            nf, S, D2 = st["nf"], st["S"], st["d2"]
            T = nf // P
            fine = 3 - s if s < 3 else 0
            coarse = 4 - s
            nchunk = (S + 511) // 512

            # B-side [4, S]: rows 0-2 = 2*xt_coarse, row 3 = -|b|^2
            b4 = abpool.tile([4, S], f32, tag="b4")
            nc.sync.dma_start(b4[0:3, :], xt[coarse][:, :])
            sqb = abpool.tile([3, S], f32, tag="sqb")
            nc.vector.tensor_tensor(sqb[:], b4[0:3, :], b4[0:3, :],
                                    op=Alu.mult)
            nc.scalar.mul(b4[0:3, :], b4[0:3, :], 2.0)
            for h0 in range(0, S, 1024):
                hw_ = min(1024, S - h0)
                pnb = ps_cb.tile([P, 1024], f32, tag="pcb")
                for c0 in range(0, hw_, 512):
                    w = min(512, hw_ - c0)
                    nc.tensor.matmul(pnb[0:1, c0:c0 + w], neg3[:],
                                     sqb[:, h0 + c0:h0 + c0 + w],
                                     start=True, stop=True)
                nc.scalar.copy(sqb[0:1, h0:h0 + hw_], pnb[0:1, :hw_])
            nc.sync.dma_start(b4[3:4, :], sqb[0:1, :])

            # A-side [4, nf]: rows 0-2 = xt_fine, row 3 = 1
            a4 = abpool.tile([4, nf], f32, tag="a4")
            nc.sync.dma_start(a4[0:3, :], xt[fine][:, :])
            nc.sync.dma_start(a4[3:4, :], onesd[:, :nf])

            # query norms + eps, row layout: point t*128+p at [p, t]
            rx = smpool.tile([P, T * 3], f32, tag="rx")
            nc.sync.dma_start(
                rx[:].rearrange("p (t c) -> p t c", c=3),
                xr[fine][:, :].rearrange("(t p) c -> p t c", p=P))
            sqa = smpool.tile([P, T * 3], f32, tag="sqa")
            nc.vector.tensor_tensor(sqa[:], rx[:], rx[:], op=Alu.mult)
            nanrm = smpool.tile([P, T], f32, tag="nanrm")
            nc.vector.tensor_reduce(
                nanrm[:], sqa[:].rearrange("p (t c) -> p t c", c=3),
                axis=Axis.X, op=Alu.add)
            nc.vector.tensor_scalar_mul(nanrm[:], nanrm[:], -1.0)

            # per-stage arrays
            maxb = smpool.tile([P, T * 8], f32, tag="maxb")
            idxb = smpool.tile([P, T * 8], u32, tag="idxb")
            d3 = smpool.tile([P, T * KNN], f32, tag="d3")
            w3 = smpool.tile([P, T * KNN], f32, tag="w3")
            wgt = smpool.tile([P, T * KNN], f32, tag="wgt")
            wsum = smpool.tile([P, T], f32, tag="wsum")

            # software-pipelined tile loop: slot i runs the front half
            # (distances, top-3, gather launch, weights, diag) for tile i and
            # the back half (PE combine, psum copies, row write) for tile
            # i-LAG, so no engine's in-order stream sits behind work whose
            # inputs (the gathers) are still in flight.
            gts, diags = {}, {}
            for i in range(T + LAG):
                if i < T:
                    t = i
                    # nd_sb holds -d in fp16: the per-query |a|^2 is folded in
                    # as an ACT bias during the PSUM drain, so small distances
                    # keep full fp16 relative precision.
                    nd_sb = ndpool.tile([P, max(S, 512)], f16, tag="nd")
                    for h0 in range(0, S, 1024):
                        hw_ = min(1024, S - h0)
                        psd = ps_d.tile([P, 1024], f32, tag="psd")
                        for c0 in range(0, hw_, 512):
                            w = min(512, hw_ - c0)
                            nc.tensor.matmul(
                                psd[:, c0:c0 + w],
                                a4[:, t * P:(t + 1) * P],
                                b4[:, h0 + c0:h0 + c0 + w],
                                start=True, stop=True)
                        nc.scalar.activation(
                            nd_sb[:, h0:h0 + hw_], psd[:, :hw_],
                            mybir.ActivationFunctionType.Identity,
                            bias=nanrm[:, t:t + 1], scale=1.0)
                    nc.vector.max(maxb[:, t * 8:(t + 1) * 8], nd_sb[:, :S])
                    nc.vector.max_index(idxb[:, t * 8:(t + 1) * 8],
                                        maxb[:, t * 8:(t + 1) * 8],
                                        nd_sb[:, :S])
                    gt = gpool.tile([P, KNN * D2], f16, tag="gt")
                    gts[t] = gt
                    for k in range(KNN):
                        nc.gpsimd.indirect_dma_start(
                            out=gt[:, k * D2:(k + 1) * D2], out_offset=None,
                            in_=gsrc[s][:, :],
                            in_offset=IndirectOffsetOnAxis(
                                ap=idxb[:, t * 8 + k:t * 8 + k + 1], axis=0))
                    # per-tile weights: maxb = -d, so d3 = -maxb + eps
                    t3 = slice(t * KNN, (t + 1) * KNN)
                    nc.vector.tensor_scalar(
                        d3[:, t3], maxb[:, t * 8:t * 8 + KNN], -1.0,
                        EPS, op0=Alu.mult, op1=Alu.add)
                    nc.vector.reciprocal(w3[:, t3], d3[:, t3])
                    nc.vector.tensor_reduce(
                        wsum[:, t:t + 1],
                        w3[:, t3].rearrange("p (o e) -> p o e", o=1),
                        axis=Axis.X, op=Alu.add)
                    nc.vector.reciprocal(wsum[:, t:t + 1], wsum[:, t:t + 1])
                    nc.vector.tensor_scalar_mul(wgt[:, t3], w3[:, t3],
                                                wsum[:, t:t + 1])
                    diag = dpool.tile([P, KNN * P], f16, tag="diag")
                    diags[t] = diag
                    for k in range(KNN):
                        nc.vector.tensor_scalar_mul(
                            diag[:, k * P:(k + 1) * P], ident[:],
                            wgt[:, t * KNN + k:t * KNN + k + 1])
                if i >= LAG:
                    t = i - LAG
                    gt, diag = gts.pop(t), diags.pop(t)
                    res = rpool.tile([P, D2], f16, tag="res")
                    for h0 in range(0, D2, 1024):
                        hw_ = min(1024, D2 - h0)
                        pcb = ps_cb.tile([P, 1024], f32, tag="pcb")
                        for c0 in range(0, hw_, 512):
                            w = min(512, hw_ - c0)
                            for k in range(KNN):
                                nc.tensor.matmul(
                                    pcb[:, c0:c0 + w],
                                    diag[:, k * P:(k + 1) * P],
                                    gt[:, k * D2 + h0 + c0:
                                       k * D2 + h0 + c0 + w],
                                    start=(k == 0), stop=(k == KNN - 1))
                        nc.scalar.copy(res[:, h0:h0 + hw_], pcb[:, :hw_])
                    if s < 3:
                        nc.sync.dma_start(
                            tsrc[s + 1][t * P:(t + 1) * P,
                                        st["cx"]:st["cx"] + D2], res[:])
                    else:
                        nc.sync.dma_start(oi[t * P:(t + 1) * P, :], res[:])
    if split_waits:
        _split_multi_waits(nc)
    return nc


def _split_multi_waits(nc):
    """This walrus build rejects instructions carrying more than one sync
    wait. Hoist extra waits into same-engine NoOps inserted just before."""
    import concourse.mybir as mybir

    n = 0
    for f in nc.m.functions:
        for bb in f.blocks:
            il = bb.instructions
            i = 0
            while i < len(il):
                inst = il[i]
                si = getattr(inst, "sync_info", None)
                ow = list(si.on_wait) if si is not None else []
                if len(ow) > 1:
                    for w in ow[:-1]:
                        nop = mybir.InstNoOp(name=f"W{n}-{inst.name}",
                                             ins=[], outs=[])
                        n += 1
                        nop.engine = inst.engine
                        nop.sync_info = mybir.SyncInfo(on_update=[],
                                                       on_wait=[w])
                        il.insert(i, nop)
                        i += 1
                    inst.sync_info = mybir.SyncInfo(
                        on_update=list(si.on_update), on_wait=[ow[-1]])
                i += 1


def _patch_tile_drain():
    """This walrus build rejects >1 sync-wait on the kernel-tail Drain; spread
    the waits across single-wait SP nops instead."""
    import concourse.mybir as mybir
    import concourse.tile as tile
    from concourse.vector_clock import ScopedClock

    if getattr(tile.TileContext, "_drain_patched", False):
        return

    def _patched(self, tick_clock, wait_clock):
        nc = self.nc
        probe = nc.sync.nop()
        wait_clock.add_sem_waits(probe.ins,
                                 ScopedClock({None: tick_clock.global_clock}))
        si = probe.ins.sync_info
        ow = list(si.on_wait) if si is not None else []
        if len(ow) > 1:
            for w in ow[1:]:
                n2 = nc.sync.nop()
                n2.ins.sync_info = mybir.SyncInfo(on_update=[], on_wait=[w])
            probe.ins.sync_info = mybir.SyncInfo(on_update=list(si.on_update),
                                                 on_wait=[ow[0]])
        nc.sync.drain()
        nc.all_engine_barrier()
        assert self.sems is not None
        popped = nc._tile_sem_poison_stack.pop()
        assert popped is self._sem_poison
        nc.clear_and_free_semaphores(list(self.sems.allocated().values()))
        nc.all_engine_barrier()

    tile.TileContext._drain_and_barrier = _patched
    tile.TileContext._drain_patched = True


def _get_program(ns, cs, n_half):
    key = (tuple(ns), tuple(cs), n_half)
    if _CACHED["key"] != key:
        _CACHED["nc"] = _build_program(ns, cs, n_half)
        _CACHED["key"] = key
    return _CACHED["nc"]


def make_core_inputs(inputs, ns, n_half, core):
    """Slice/transform full inputs for one core (b = core//2, half = core%2)."""
    b, h = core // 2, core % 2
    d = {}
    x0h = np.ascontiguousarray(
        np.asarray(inputs["xyz0"])[b, h * n_half:(h + 1) * n_half])
    d["xt0"] = np.ascontiguousarray(x0h.T)
    d["xr0"] = x0h
    for i in range(1, 5):
        xi = np.ascontiguousarray(np.asarray(inputs[f"xyz{i}"])[b])
        d[f"xt{i}"] = np.ascontiguousarray(xi.T)
        d[f"xr{i}"] = xi
        d[f"ft{i}"] = np.ascontiguousarray(
            np.asarray(inputs[f"x{i}"])[b].T.astype(np.float16))
    d["ones"] = np.ones((1, max(n_half, ns[1])), np.float32)
    return d


def kernel(**inputs):
    from concourse.bass_utils import run_bass_kernel_spmd

    ns, cs = NS, CS
    n_half = ns[0] // 2
    nc = _get_program(ns, cs, n_half)

    in_maps = [make_core_inputs(inputs, ns, n_half, c) for c in range(8)]
    res = run_bass_kernel_spmd(nc, in_maps, core_ids=list(range(8)))

    dout = sum(cs)
    out = np.empty((B, dout, ns[0]), np.float32)
    out[:, :cs[0], :] = np.asarray(inputs["x0"])
    for c in range(8):
        b, h = c // 2, c % 2
        out[b, cs[0]:, h * n_half:(h + 1) * n_half] = \
            res.results[c]["oi"].astype(np.float32).T
    return out
